# revision 57
# baseline (speedup 1.0000x reference)
"""Trainium2 Bass kernel for the sparse-attention ('interact' mask) transformer block.

Reference computation (B=4, N=1569, C=768, H=12, d=64, Dff=3072, F=9):
    h   = LN(x);  qkv = h @ qkv_w.T;  sparse attention (spatial rows attend
    only to the 9 temporal tokens, temporal rows attend to the 1560 spatial
    tokens, CLS also to itself);  out = attn @ proj_w.T + proj_b;
    return out + MLP(LN(out))

Sharding: 8 cores = 4 batches x 2 halves. Each core owns one batch's half of
the 1560 spatial tokens (780) plus a replicated copy of the 9 temporal
tokens; local token layout is [780 spatial | 9 temporal | 1 pad].  The only
communication is one pairwise AllReduce(add) of flash-style partial softmax
stats packed as a single [10,768] tile.

Structure (~224us, vs the 255us v2 baseline):
- x and the output travel as bf16 (halves startup/tail DMA); all 24 fc2
  slabs are SBUF-resident from early on (the v2 175us stall was fc2 slabs
  re-loading into the proj slots); fc1 slabs overwrite the qkv slabs
  in-place, chunked per 768-col block so each block only WARs the qkv part
  it replaces.
- The [768x768] proj GEMM is folded into the attention output: since
  attn_out_spatial = vtmp_bd.T @ (p1*rb), projout = Wt.T @ (p1*rb) with
  Wt = bdT.T @ projWt, a 108-contraction GEMM (saves ~22k PE cycles and
  the attnout evictions). proj_w @ beta_v is folded into the proj bias.
- GEMM order q -> v -> k releases the fc1/fc2 slab WAR gates early; S1/S2T
  run after k; O2 is one [108,768] accumulation + mask-collapse instead of
  84 tiny per-head matmuls; kbd/qbd temporal slices are read straight from
  PSUM so S1/S2T never wait the full q/k evictions.
- fc1/fc2 split into T1 (cols 0:512) / T2 (512:790) passes ordered
  fc1T1 -> fc2T1 -> fixup -> fc1T2 -> fc2T2 so the AllReduce only gates
  the last ~35us of PE work; LN applies read the broadcast mu/alpha
  directly from PSUM; small junk-matmul blocks cover the DMA-boot and LN
  chain-latency windows so the PE HAM clock gate stays at 2.4GHz.

LN gamma/beta are folded host-side: gamma into the weight matrices, beta
into effective output biases (qkv bias is applied on q/k evictions; the v
bias is added post-attention, which is exact because softmax weights sum
to 1).
"""

import numpy as np
import sys
from contextlib import ExitStack

sys.path.insert(0, '/opt/trn_rl_repo')

import concourse.bass as bass
import concourse.bacc as bacc
import concourse.tile as tile
from concourse import mybir
from concourse.bass_utils import run_bass_kernel_spmd

# ---------------- problem constants (hardcoded per contract) ----------------
B, N, C = 4, 1569, 768
H, D = 12, 64
F = 9                    # temporal tokens (CLS + 8 frames)
DFF = 4 * C              # 3072
NSP = N - F              # 1560 spatial tokens
SPH = NSP // 2           # 780 spatial tokens per core
T = SPH + F + 1          # 790 local cols: [780 spatial | 9 temporal | 1 pad]
NCH = C // 128           # 6 feature chunks
NFF = DFF // 128         # 24 hidden chunks
NTB = (T + 127) // 128   # 7 token blocks (last = 22 rows)
SCALE = D ** -0.5
EPS = 1e-5

FP32 = mybir.dt.float32
BF16 = mybir.dt.bfloat16

TF = [(0, 512), (512, T)]        # full-width tiles (LN1, qkv, S1)
TS = [(0, 512), (512, SPH)]      # spatial-only tiles (O1, proj, LN2)
TX = (SPH, T)                    # temporal+pad fixup tile (10 cols)

# consts blob (bf16) column layout
CB_ONES = 0
CB_HSEL = 1            # headsel rows 0:12, cols 1:769
CB_BD9 = 769           # bd9 rows 0:108, cols 769:781
CB_BD9T = 781          # bd9T rows 0:12, cols 781:889
CB_MASK = 889          # [1,108] CLS-self mask row (row 0), cols 889:997
CB_ONESROW = 997       # row 0 all-ones, cols 997:1125
CB_ID9 = 1128          # [9,9] bf16 identity, cols 1128:1137
CB_CMASK = 1140        # [108,768] head column mask, cols 1140:1908
CB_SELT = 1908         # [108,9] selT[h*F+j, j]=1, cols 1908:1917
CB_W = 1920

# bias blob (fp32) column layout
BB_QB, BB_KB, BB_VB, BB_PB, BB_F1B, BB_F2B = 0, 6, 12, 18, 24, 48
BB_EPS = 54
BB_S2B = 55
BB_W = 56


def build_kernel():
    nc = bacc.Bacc("TRN2", target_bir_lowering=False, debug=False,
                   num_devices=8)

    # ---------------- DRAM I/O ----------------
    xT = nc.dram_tensor("xT", [C, T], BF16, kind="ExternalInput")
    qkvWt = nc.dram_tensor("qkvWt", [C, 3 * C], BF16, kind="ExternalInput")
    projWt = nc.dram_tensor("projWt", [C, C], BF16, kind="ExternalInput")
    fc1Wt = nc.dram_tensor("fc1Wt", [C, DFF], BF16, kind="ExternalInput")
    fc2Wt = nc.dram_tensor("fc2Wt", [DFF, C], BF16, kind="ExternalInput")
    cblob = nc.dram_tensor("cblob", [128, CB_W], BF16, kind="ExternalInput")
    bblob = nc.dram_tensor("bblob", [128, BB_W], FP32, kind="ExternalInput")
    ident = nc.dram_tensor("ident", [16, 16], FP32, kind="ExternalInput")

    outT = nc.dram_tensor("outT", [C, T], BF16, kind="ExternalOutput")

    with tile.TileContext(nc) as tc, ExitStack() as ctx:
        act = ctx.enter_context(tc.tile_pool(name="act", bufs=1))
        big = ctx.enter_context(tc.tile_pool(name="big", bufs=1))
        wq = ctx.enter_context(tc.tile_pool(name="wq", bufs=1))
        wp = ctx.enter_context(tc.tile_pool(name="wp", bufs=1))
        wpj = ctx.enter_context(tc.tile_pool(name="wpj", bufs=1))
        small = ctx.enter_context(tc.tile_pool(name="small", bufs=1))
        rows = ctx.enter_context(tc.tile_pool(name="rows", bufs=1))
        scr = ctx.enter_context(tc.tile_pool(name="scr", bufs=1))
        stage = ctx.enter_context(tc.tile_pool(name="stage", bufs=4))
        psmm = ctx.enter_context(tc.tile_pool(name="psmm", bufs=4, space="PSUM"))
        psst = ctx.enter_context(tc.tile_pool(name="psst", bufs=2, space="PSUM"))
        pso2 = ctx.enter_context(tc.tile_pool(name="pso2", bufs=2, space="PSUM"))
        dram = ctx.enter_context(tc.tile_pool(name="dram", bufs=1, space="DRAM"))

        # PE keep-warm filler: blocks of dependency-free junk matmuls emitted
        # into latency windows (DMA boot, LN chain hops) so the HAM clock
        # gate never sees a >3.4us idle window and re-throttles to 1.2GHz.
        junk = small.tile([128, 512], BF16, tag="junk", name="junk")
        nc.vector.memset(junk[:], 0)

        def warm(n):
            psj = psmm.tile([128, 512], FP32, tag="mm", name="warm")
            for _ in range(n):
                nc.tensor.matmul(psj[:], junk[:, 0:128], junk[:],
                                 start=True, stop=True)

        # ---------------- input DMAs, in queue-priority order ----------------
        cb = small.tile([128, CB_W], BF16, tag="cb", name="cb")
        nc.sync.dma_start(cb[:], cblob[:])
        bb = small.tile([128, BB_W], FP32, tag="bb", name="bb")
        nc.sync.dma_start(bb[:], bblob[:])

        ones = cb[:, CB_ONES:CB_ONES + 1]                 # [128,1] bf16
        onesrow_b = cb[0:1, CB_ONESROW:CB_ONESROW + 128]  # [1,128] bf16

        x_t = [act.tile([128, T], BF16, tag=f"x{ci}", name=f"x{ci}")
               for ci in range(NCH)]
        for ci in range(NCH):
            nc.sync.dma_start(x_t[ci][:], xT[ci * 128:(ci + 1) * 128, :])

        # qkv weights: full-resident 6 x [128, 3072]; the q part lands as 36
        # co-major blocks so the q GEMM starts as soon as its first column
        # blocks arrive, then k/v as whole slabs.
        wq_t = [wq.tile([128, DFF], BF16, tag=f"wq{ci}", name=f"wq{ci}")
                for ci in range(NCH)]
        for half_ in range(2):     # q part in half-slabs: early start, few descs
            for ci in range(NCH):
                nc.sync.dma_start(
                    wq_t[ci][:, half_ * 384:(half_ + 1) * 384],
                    qkvWt[ci * 128:(ci + 1) * 128, half_ * 384:(half_ + 1) * 384])
        for part in (2, 1):    # v-part before k-part: GEMM order is q, v, k
            for ci in range(NCH):
                nc.sync.dma_start(
                    wq_t[ci][:, part * C:(part + 1) * C],
                    qkvWt[ci * 128:(ci + 1) * 128, part * C:(part + 1) * C])
        # proj weights (own pool now)
        wp_t = [wpj.tile([128, C], BF16, tag=f"wpj{i}", name=f"wpj{i}")
                for i in range(NCH)]
        for ci in range(NCH):
            nc.sync.dma_start(wp_t[ci][:], projWt[ci * 128:(ci + 1) * 128, :])
        # fc2 slab tiles (declared here; DMAs are emitted after the v GEMM so
        # they sit behind the startup-critical traffic in the sync queue)
        wf2_t = [wp.tile([128, C], BF16, tag=f"wp{i}", name=f"wf2{i}")
                 for i in range(NFF)]
        # fc1 slabs reuse the qkv tiles in place; chunked by 768-col blocks so
        # block b only WARs on the qkv part it overwrites (b0<-q, b1<-k,
        # b2<-v; b3 lands on never-written cols and can load immediately).
        def wf1_load(blk, eng=None):
            for ci in range(NCH):
                (eng or nc.sync).dma_start(
                    wq_t[ci][:, blk * C:(blk + 1) * C],
                    fc1Wt[ci * 128:(ci + 1) * 128, blk * C:(blk + 1) * C])

        wf1_load(3)
        id_t = small.tile([16, 16], FP32, tag="id", name="id")
        nc.sync.dma_start(id_t[:], ident[:])

        # early memsets (no deps; keeps them off the critical path)
        kbd = [small.tile([128, H * F], BF16, tag=f"kbd{ci}", name=f"kbd{ci}")
               for ci in range(NCH)]
        qbd = [small.tile([128, H * F], BF16, tag=f"qbd{ci}", name=f"qbd{ci}")
               for ci in range(NCH)]
        # transposed block-diag temporal v [c', (h,j)] per feature chunk, for
        # the fused proj weight Wt = vtmp_bdT.T @ projWt
        bdT = [small.tile([128, H * F], BF16, tag=f"bdT{ci}", name=f"bdT{ci}")
               for ci in range(NCH)]
        for ci in range(NCH):
            nc.vector.memset(kbd[ci][:], 0)
            nc.vector.memset(qbd[ci][:], 0)
            nc.vector.memset(bdT[ci][:], 0)

        # =========================================================
        # layernorm helpers
        # =========================================================
        def ln_chain(ps_sum, ps_sq, mu_t, al_t, t0, t1):
            w = t1 - t0
            musq = rows.tile([1, 512], FP32, tag="row", name="musq", bufs=2)
            ex2 = rows.tile([1, 512], FP32, tag="row", name="ex2", bufs=2)
            nc.scalar.activation(mu_t[:, t0:t1], ps_sum[:, :w],
                                 mybir.ActivationFunctionType.Identity,
                                 scale=1.0 / C)
            nc.scalar.activation(musq[:, :w], ps_sum[:, :w],
                                 mybir.ActivationFunctionType.Square,
                                 scale=1.0 / C)
            nc.scalar.activation(ex2[:, :w], ps_sq[:, :w],
                                 mybir.ActivationFunctionType.Identity,
                                 scale=1.0 / C)
            nc.vector.tensor_sub(ex2[:, :w], ex2[:, :w], musq[:, :w])
            nc.scalar.activation(musq[:, :w], ex2[:, :w],
                                 mybir.ActivationFunctionType.Sqrt,
                                 bias=bb[0:1, BB_EPS:BB_EPS + 1])
            nc.vector.reciprocal_approx_fast(ex2[:, :w], musq[:, :w])
            nc.scalar.copy(al_t[:, t0:t1], ex2[:, :w])

        def ln_bcast(mu_t, al_t, t0, t1):
            w = t1 - t0
            out = []
            for srow in (mu_t, al_t):
                psb = psmm.tile([128, 512], FP32, tag="mm", name="lnbc")
                nc.tensor.matmul(psb[:, :w], onesrow_b, srow[:, t0:t1],
                                 start=True, stop=True)
                out.append(psb)
            return out

        def ln_apply(src, dst, psmu, psal, t0, t1):
            w = t1 - t0
            for ci in range(NCH):
                s = scr.tile([128, 512], FP32, tag="scr", name="lnscr")
                nc.vector.tensor_sub(s[:, :w], src[ci][:, t0:t1],
                                     psmu[:, :w])
                nc.vector.tensor_mul(dst[ci][:, t0:t1], s[:, :w],
                                     psal[:, :w])

        # =========================================================
        # STAGE A: LN1 (full 790 cols)
        # =========================================================
        warm(12)   # cover the DMA-boot dead zone
        sq = [act.tile([128, T], BF16, tag=f"k{ci}", name=f"sq{ci}")
              for ci in range(NCH)]
        for ci in range(NCH):
            nc.vector.tensor_mul(sq[ci][:], x_t[ci][:], x_t[ci][:])
        mu_t = rows.tile([1, T], BF16, tag="mu", name="mu1")
        al_t = rows.tile([1, T], BF16, tag="al", name="al1")
        stats = []
        for (t0, t1) in TF:
            w = t1 - t0
            ps_sum = psst.tile([12, 512], FP32, tag="st", name="sum")
            for ci in range(NCH):
                nc.tensor.matmul(ps_sum[0:1, :w], ones,
                                 x_t[ci][:, t0:t1],
                                 start=(ci == 0), stop=(ci == NCH - 1))
            ps_sq = psst.tile([12, 512], FP32, tag="st", name="sumsq")
            for ci in range(NCH):
                nc.tensor.matmul(ps_sq[0:1, :w], ones, sq[ci][:, t0:t1],
                                 start=(ci == 0), stop=(ci == NCH - 1))
            stats.append((ps_sum, ps_sq))
        h_t = [act.tile([128, T], BF16, tag=f"h{ci}", name=f"h{ci}")
               for ci in range(NCH)]
        for si, ((t0, t1), (ps_sum, ps_sq)) in enumerate(zip(TF, stats)):
            ln_chain(ps_sum[0:1], ps_sq[0:1], mu_t, al_t, t0, t1)
            warm(12 if si == 0 else 6)   # PE filler while the chain hops
            psmu, psal = ln_bcast(mu_t, al_t, t0, t1)
            ln_apply(x_t, h_t, psmu, psal, t0, t1)

        # =========================================================
        # STAGE B: q, k feature-major [C, T] bf16 (+ folded LN-beta bias)
        # =========================================================
        q_t = [act.tile([128, T], BF16, tag=f"q{ci}", name=f"q{ci}")
               for ci in range(NCH)]
        k_t = [act.tile([128, T], BF16, tag=f"k{ci}", name=f"k{ci}")
               for ci in range(NCH)]

        def qk_gemm(qk, dst, bbc):
            for co in range(NCH):
                pss = [psmm.tile([128, 512], FP32, tag="mm", name="mm")
                       for _ in TF]
                for ci in range(NCH):
                    for (t0, t1), ps in zip(TF, pss):
                        nc.tensor.matmul(
                            ps[:, :t1 - t0],
                            wq_t[ci][:, qk * C + co * 128: qk * C + (co + 1) * 128],
                            h_t[ci][:, t0:t1],
                            start=(ci == 0), stop=(ci == NCH - 1))
                # block-diag temporal slices straight from PSUM (with bias)
                # so S1/S2T never wait on the full-tile evictions
                bd = kbd if qk == 1 else qbd
                for hh in (2 * co, 2 * co + 1):
                    po = (hh % 2) * 64
                    nc.vector.tensor_scalar_add(
                        bd[co][po:po + 64, hh * F:(hh + 1) * F],
                        pss[1][po:po + 64, SPH - 512:SPH - 512 + F],
                        bb[po:po + 64, bbc + co:bbc + co + 1])
                for (t0, t1), ps in zip(TF, pss):
                    if qk == 1:   # k evictions on DVE to unload ACT
                        nc.vector.tensor_scalar_add(
                            dst[co][:, t0:t1], ps[:, :t1 - t0],
                            bb[:, bbc + co:bbc + co + 1])
                    else:
                        nc.scalar.activation(
                            dst[co][:, t0:t1], ps[:, :t1 - t0],
                            mybir.ActivationFunctionType.Identity,
                            bias=bb[:, bbc + co:bbc + co + 1])

        qk_gemm(0, q_t, BB_QB)
        wf1_load(0)

        # =========================================================
        # STAGE C: v token-major [T, C] bf16, temporal block FIRST.
        # Before the k GEMM so the v-part WAR gate on the fc1/fc2 slab
        # traffic releases early.
        # =========================================================
        v_t = [big.tile([128, C], BF16, tag=f"v{tb}", name=f"v{tb}")
               for tb in range(NTB)]
        vtmp9 = small.tile([F, C], BF16, tag="vtmp9", name="vtmp9")
        for tb in [6, 0, 1, 2, 3, 4, 5]:
            p0, p1_ = tb * 128, min((tb + 1) * 128, T)
            rr = p1_ - p0
            pss = [psmm.tile([128, 512], FP32, tag="mm", name="mmv")
                   for _ in range(2)]
            for ci in range(NCH):
                for (c0, c1), ps in zip(((0, 512), (512, C)), pss):
                    nc.tensor.matmul(ps[:rr, :c1 - c0],
                                     h_t[ci][:, p0:p1_],
                                     wq_t[ci][:, 2 * C + c0: 2 * C + c1],
                                     start=(ci == 0), stop=(ci == NCH - 1))
            for (c0, c1), ps in zip(((0, 512), (512, C)), pss):
                nc.vector.tensor_copy(v_t[tb][:rr, c0:c1], ps[:rr, :c1 - c0])
            if tb == 6:
                # temporal v rows -> partitions 0..8 (one cross-partition DMA
                # on the gpsimd queue; lands while the rest of v computes)
                nc.gpsimd.dma_start(vtmp9[:], v_t[6][12:12 + F, :])
        wf1_load(2)
        for i in range(NFF):
            nc.sync.dma_start(wf2_t[i][:], fc2Wt[i * 128:(i + 1) * 128, :])

        qk_gemm(1, k_t, BB_KB)
        wf1_load(1)

        # =========================================================
        # STAGE D: S1 (all queries vs 9 temporal keys) and S2T
        # =========================================================
        p1 = small.tile([H * F, T], BF16, tag="p1", name="p1")
        ps_s1 = [psmm.tile([128, 512], FP32, tag="mm", name="mms1")
                 for _ in TF]
        for ci in range(NCH):
            for (t0, t1), ps in zip(TF, ps_s1):
                nc.tensor.matmul(ps[:H * F, :t1 - t0], kbd[ci][:],
                                 q_t[ci][:, t0:t1],
                                 start=(ci == 0), stop=(ci == NCH - 1))
        for (t0, t1), ps in zip(TF, ps_s1):
            nc.scalar.activation(p1[:, t0:t1], ps[:H * F, :t1 - t0],
                                 mybir.ActivationFunctionType.Exp, scale=SCALE)

        # S2T: temporal queries vs all local keys, token-major p2 [T, 108]
        p2 = [small.tile([128, H * F], BF16, tag=f"p2{tb}", name=f"p2{tb}")
              for tb in range(NTB)]
        for tb in range(NTB):
            p0, p1_ = tb * 128, min((tb + 1) * 128, T)
            rr = p1_ - p0
            ps = psmm.tile([128, 512], FP32, tag="mm", name="mms2")
            for ci in range(NCH):
                nc.tensor.matmul(ps[:rr, :H * F],
                                 k_t[ci][:, p0:p1_], qbd[ci][:],
                                 start=(ci == 0), stop=(ci == NCH - 1))
            # the last block holds the 9 temporal keys + pad at partitions
            # 12..21: a -1e4 pre-exp bias on partitions 13..21 zeroes them
            nc.scalar.activation(p2[tb][:rr, :], ps[:rr, :H * F],
                                 mybir.ActivationFunctionType.Exp, scale=SCALE,
                                 bias=(bb[0:rr, BB_S2B:BB_S2B + 1] if tb == 6
                                       else 0.0))
        # CLS-key row (partition 12): keep only the CLS self-term, and only on
        # even cores (DVE cannot address partition 12, so bounce via DMA)
        e00tmp = small.tile([1, H * F], BF16, tag="e00t", name="e00t")
        nc.gpsimd.dma_start(e00tmp[:], p2[6][12:13, :])
        nc.vector.tensor_mul(e00tmp[:], e00tmp[:],
                             cb[0:1, CB_MASK:CB_MASK + H * F])
        nc.gpsimd.dma_start(p2[6][12:13, :], e00tmp[:])

        # =========================================================
        # STAGE E: softmax denominators + O2/l2 partials + AllReduce
        # =========================================================
        # lsp[h,t] = sum_j p1[(h,j),t] ; rlsp = 1/lsp
        lsp = small.tile([12, T], BF16, tag="lsp", name="lsp")
        for (t0, t1) in TF:
            ps = psst.tile([12, 512], FP32, tag="st", name="lspps")
            nc.tensor.matmul(ps[:, :t1 - t0],
                             cb[0:H * F, CB_BD9:CB_BD9 + 12],
                             p1[:, t0:t1], start=True, stop=True)
            s = scr.tile([128, 512], FP32, tag="scr", name="rlscr")
            nc.vector.reciprocal_approx_fast(s[0:12, :t1 - t0], ps[:, :t1 - t0])
            nc.scalar.copy(lsp[:, t0:t1], s[0:12, :t1 - t0])

        # broadcast rlsp over (h,j) rows -> rb [108, T] bf16 (for O1 weights)
        rb = small.tile([H * F, T], BF16, tag="rb", name="rb")
        for (t0, t1) in TF:
            ps = psmm.tile([128, 512], FP32, tag="mm", name="mmrb")
            nc.tensor.matmul(ps[:H * F, :t1 - t0],
                             cb[0:12, CB_BD9T:CB_BD9T + H * F],
                             lsp[:, t0:t1], start=True, stop=True)
            nc.scalar.copy(rb[:, t0:t1], ps[:H * F, :t1 - t0])

        # build the transposed block-diag temporal v for the fused proj
        for ci in range(NCH):
            pst = psmm.tile([128, 512], BF16, tag="mm", name="vtr")
            nc.tensor.transpose(pst[:128, :F],
                                vtmp9[:, ci * 128:(ci + 1) * 128],
                                cb[0:F, CB_ID9:CB_ID9 + F])
            for half_ in range(2):
                hh = 2 * ci + half_
                nc.vector.tensor_copy(
                    bdT[ci][half_ * 64:half_ * 64 + 64,
                            hh * F:(hh + 1) * F],
                    pst[half_ * 64:half_ * 64 + 64, :F])

        # =========================================================
        # STAGE F: fused proj weight Wt[hj,c] = sum_c' bdT[c',hj] projWt[c',c]
        # (projout_spatial = proj(attn_out) = Wt.T @ (p1*rb), so the whole
        #  [768x768] proj GEMM collapses into a 108-contraction one)
        # =========================================================
        wt_sb = small.tile([H * F, C], BF16, tag="wtsb", name="wtsb")
        psw = [psmm.tile([128, 512], FP32, tag="mm", name="mmwt")
               for _ in range(2)]
        for ci in range(NCH):
            for (c0, c1), ps in zip(((0, 512), (512, C)), psw):
                nc.tensor.matmul(ps[:H * F, :c1 - c0], bdT[ci][:],
                                 wp_t[ci][:, c0:c1],
                                 start=(ci == 0), stop=(ci == NCH - 1))
        for (c0, c1), ps in zip(((0, 512), (512, C)), psw):
            nc.scalar.copy(wt_sb[:, c0:c1], ps[:H * F, :c1 - c0])

        # attnout tiles only carry the temporal fixup columns now
        attnout = [act.tile([128, T], BF16, tag=f"x{ci}", name=f"attn{ci}")
                   for ci in range(NCH)]
        for ci in range(NCH):
            nc.vector.memset(attnout[ci][:, SPH + F:T], 0)  # pad col stays 0

        # =========================================================
        # STAGE G: projout spatial (+ folded proj_b + proj_w@beta_v bias)
        # =========================================================
        for (t0, t1) in TS:
            nc.vector.tensor_mul(p1[:, t0:t1], p1[:, t0:t1], rb[:, t0:t1])
        projout = [act.tile([128, T], BF16, tag=f"h{ci}", name=f"po{ci}")
                   for ci in range(NCH)]
        sq2 = [act.tile([128, T], BF16, tag=f"k{ci}", name=f"sq2{ci}")
               for ci in range(NCH)]
        for co in range(NCH):
            pss = [psmm.tile([128, 512], FP32, tag="mm", name="mmpj")
                   for _ in TS]
            for (t0, t1), ps in zip(TS, pss):
                nc.tensor.matmul(ps[:, :t1 - t0],
                                 wt_sb[:, co * 128:(co + 1) * 128],
                                 p1[:, t0:t1], start=True, stop=True)
            for (t0, t1), ps in zip(TS, pss):
                nc.scalar.activation(projout[co][:, t0:t1], ps[:, :t1 - t0],
                                     mybir.ActivationFunctionType.Identity,
                                     bias=bb[:, BB_PB + co:BB_PB + co + 1])
                nc.vector.tensor_mul(sq2[co][:, t0:t1], projout[co][:, t0:t1],
                                     projout[co][:, t0:t1])

        # =========================================================
        # STAGE G2: l2/O2 partials + AllReduce (PE work fits in the window
        # where ACT/DVE evict projout and run the LN2 stats chain)
        # =========================================================
        l2row = small.tile([1, H * F], FP32, tag="l2", name="l2")
        ps_l2 = psst.tile([12, 512], FP32, tag="st", name="l2ps")
        for tb in range(NTB):
            p0, p1_ = tb * 128, min((tb + 1) * 128, T)
            nc.tensor.matmul(ps_l2[0:1, :H * F],
                             cb[0:p1_ - p0, CB_ONES:CB_ONES + 1],
                             p2[tb][:p1_ - p0, :],
                             start=(tb == 0), stop=(tb == NTB - 1))
        nc.scalar.copy(l2row[:], ps_l2[0:1, :H * F])

        # O2 partial [9, 768]: one [108,768] accumulation over token blocks,
        # then a mask-and-collapse (16 wide matmuls instead of 84 tiny ones)
        o2bd_ps = [psmm.tile([128, 512], FP32, tag="mm", name="o2bd")
                   for _ in range(2)]
        for tb in range(NTB):
            p0, p1_ = tb * 128, min((tb + 1) * 128, T)
            for (c0, c1), ps in zip(((0, 512), (512, C)), o2bd_ps):
                nc.tensor.matmul(ps[:H * F, :c1 - c0],
                                 p2[tb][:p1_ - p0, :],
                                 v_t[tb][:p1_ - p0, c0:c1],
                                 start=(tb == 0), stop=(tb == NTB - 1))
        o2bd_sb = small.tile([H * F, C], BF16, tag="o2bd", name="o2bd")
        for (c0, c1), ps in zip(((0, 512), (512, C)), o2bd_ps):
            nc.vector.tensor_mul(o2bd_sb[:, c0:c1], ps[:H * F, :c1 - c0],
                                 cb[0:H * F, CB_CMASK + c0:CB_CMASK + c1])
        o2 = small.tile([F, C], FP32, tag="o2", name="o2")
        for (c0, c1) in ((0, 512), (512, C)):
            pso = pso2.tile([F, 512], FP32, tag="o2", name="o2ps")
            nc.tensor.matmul(pso[:, :c1 - c0],
                             cb[0:H * F, CB_SELT:CB_SELT + F],
                             o2bd_sb[:, c0:c1], start=True, stop=True)
            nc.scalar.copy(o2[:, c0:c1], pso[:, :c1 - c0])

        # ---- single pairwise AllReduce of packed (o2 | l2), fully async ----
        cc_in = dram.tile([F + 1, C], FP32, tag="cc_in", name="cc_in")
        cc_out = dram.tile([F + 1, C], FP32, tag="cc_out", name="cc_out")
        # on the gpsimd queue: must not sit behind the weight-slab traffic
        nc.gpsimd.dma_start(cc_in[0:F, :], o2[:])
        nc.gpsimd.dma_start(cc_in[F:F + 1, 0:H * F], l2row[:])
        nc.gpsimd.collective_compute(
            "AllReduce", mybir.AluOpType.add,
            replica_groups=[[0, 1], [2, 3], [4, 5], [6, 7]],
            ins=[cc_in.opt()], outs=[cc_out.opt()])
        o2m, l2m = o2, l2row
        nc.gpsimd.dma_start(o2m[:], cc_out[0:F, :])
        nc.gpsimd.dma_start(l2m[:], cc_out[F:F + 1, 0:H * F])

        # =========================================================
        # STAGE H: LN2 (spatial) + fc1-T1
        # =========================================================
        mu2 = rows.tile([1, T], BF16, tag="mu", name="mu2")
        al2 = rows.tile([1, T], BF16, tag="al", name="al2")
        h2 = [act.tile([128, T], BF16, tag=f"q{ci}", name=f"h2{ci}")
              for ci in range(NCH)]
        hid = [big.tile([128, T], BF16, tag=f"hid{i}", name=f"hid{i}")
               for i in range(NFF)]

        def fc1_tile(t0, t1):
            w = t1 - t0
            for fo in range(NFF):
                ps = psmm.tile([128, 512], FP32, tag="mm", name="mmf1")
                for ci in range(NCH):
                    nc.tensor.matmul(
                        ps[:, :w],
                        wq_t[ci][:, fo * 128:(fo + 1) * 128],
                        h2[ci][:, t0:t1],
                        start=(ci == 0), stop=(ci == NCH - 1))
                nc.scalar.activation(hid[fo][:, t0:t1], ps[:, :w],
                                     mybir.ActivationFunctionType.Gelu,
                                     bias=bb[:, BB_F1B + fo:BB_F1B + fo + 1])

        stats2 = []
        for (t0, t1) in TS:
            w = t1 - t0
            ps_sum = psst.tile([12, 512], FP32, tag="st", name="sum2")
            for ci in range(NCH):
                nc.tensor.matmul(ps_sum[0:1, :w], ones,
                                 projout[ci][:, t0:t1],
                                 start=(ci == 0), stop=(ci == NCH - 1))
            ps_sq = psst.tile([12, 512], FP32, tag="st", name="sumsq2")
            for ci in range(NCH):
                nc.tensor.matmul(ps_sq[0:1, :w], ones, sq2[ci][:, t0:t1],
                                 start=(ci == 0), stop=(ci == NCH - 1))
            stats2.append((ps_sum, ps_sq))
        (t0, t1) = TS[0]
        ln_chain(stats2[0][0][0:1], stats2[0][1][0:1], mu2, al2, t0, t1)
        psmu, psal = ln_bcast(mu2, al2, t0, t1)
        ln_apply(projout, h2, psmu, psal, t0, t1)
        # T2 chain early (ACT runs it during fc1-T1); its PE broadcast is
        # emitted after fc1-T1 so the PE queue never head-blocks on it
        (t0, t1) = TS[1]
        ln_chain(stats2[1][0][0:1], stats2[1][1][0:1], mu2, al2, t0, t1)
        fc1_tile(*TS[0])
        # fixup normalizer prep runs on DVE/gpsimd while fc1-T1 owns the PE
        nc.vector.reciprocal_approx_fast(l2m[:], l2m[:])
        rl2hj = small.tile([12, 10], FP32, tag="rl2hj", name="rl2hj")
        nc.vector.memset(rl2hj[:], 0)
        for hh in range(H):
            nc.gpsimd.dma_start(rl2hj[hh:hh + 1, 0:F],
                                l2m[:, hh * F:(hh + 1) * F])
        rl2hj_bf = small.tile([12, 10], BF16, tag="rl2hjbf", name="rl2hjbf")
        nc.scalar.copy(rl2hj_bf[:], rl2hj[:])
        psmu, psal = ln_bcast(mu2, al2, t0, t1)
        ln_apply(projout, h2, psmu, psal, t0, t1)

        # =========================================================
        # STAGE I: fc2-T1 (cols 0:512) -- independent of the fixup
        # =========================================================
        def fc2_tile(t0, t1, cos=range(NCH)):
            w = t1 - t0
            for co in cos:
                ps = psmm.tile([128, 512], FP32, tag="mm", name="mmf2")
                for ci in range(NFF):
                    nc.tensor.matmul(ps[:, :w],
                                     wf2_t[ci][:, co * 128:(co + 1) * 128],
                                     hid[ci][:, t0:t1],
                                     start=(ci == 0), stop=(ci == NFF - 1))
                st = stage.tile([128, 512], BF16, tag="out", name="out")
                nc.scalar.activation(st[:, :w], ps[:, :w],
                                     mybir.ActivationFunctionType.Identity,
                                     bias=bb[:, BB_F2B + co:BB_F2B + co + 1])
                nc.vector.tensor_add(st[:, :w], st[:, :w],
                                     projout[co][:, t0:t1])
                nc.sync.dma_start(outT[co * 128:(co + 1) * 128, t0:t1],
                                  st[:, :w])

        fc2_tile(0, 512)

        # =========================================================
        # STAGE J: temporal fixup (consumes the AllReduce)
        # =========================================================
        # attnout temporal cols: transpose o2m, scale by 1/l2, add v-bias
        for ci in range(NCH):
            pst = psmm.tile([128, 512], FP32, tag="mm", name="mmtr")
            nc.tensor.transpose(pst[:128, :F],
                                o2m[:, ci * 128:(ci + 1) * 128],
                                id_t[:F, :F])
            psr = psmm.tile([128, 512], FP32, tag="mm", name="mmrl2")
            nc.tensor.matmul(psr[:, :10],
                             cb[0:12, CB_HSEL + ci * 128:CB_HSEL + (ci + 1) * 128],
                             rl2hj_bf[:], start=True, stop=True)
            rbc = scr.tile([128, 512], FP32, tag="scr", name="rbc")
            nc.scalar.copy(rbc[:, :F], psr[:, :F])
            nc.vector.tensor_mul(attnout[ci][:, SPH:SPH + F], pst[:128, :F],
                                 rbc[:, :F])
            # no v-bias here: proj_w @ beta_v is folded into the proj bias
        # proj on the 10 temporal+pad cols
        (t0, t1) = TX
        for co in range(NCH):
            ps = psmm.tile([128, 512], FP32, tag="mm", name="mmpjf")
            for ci in range(NCH):
                nc.tensor.matmul(ps[:, :t1 - t0],
                                 wp_t[ci][:, co * 128:(co + 1) * 128],
                                 attnout[ci][:, t0:t1],
                                 start=(ci == 0), stop=(ci == NCH - 1))
            nc.scalar.activation(projout[co][:, t0:t1], ps[:, :t1 - t0],
                                 mybir.ActivationFunctionType.Identity,
                                 bias=bb[:, BB_PB + co:BB_PB + co + 1])
            nc.vector.tensor_mul(sq2[co][:, t0:t1], projout[co][:, t0:t1],
                                 projout[co][:, t0:t1])
        # LN2 on the 10 temporal+pad cols
        ps_sum = psst.tile([12, 512], FP32, tag="st", name="sumf")
        for ci in range(NCH):
            nc.tensor.matmul(ps_sum[0:1, :t1 - t0], ones,
                             projout[ci][:, t0:t1],
                             start=(ci == 0), stop=(ci == NCH - 1))
        ps_sq = psst.tile([12, 512], FP32, tag="st", name="sumsqf")
        for ci in range(NCH):
            nc.tensor.matmul(ps_sq[0:1, :t1 - t0], ones, sq2[ci][:, t0:t1],
                             start=(ci == 0), stop=(ci == NCH - 1))
        ln_chain(ps_sum[0:1], ps_sq[0:1], mu2, al2, t0, t1)
        psmu, psal = ln_bcast(mu2, al2, t0, t1)
        ln_apply(projout, h2, psmu, psal, t0, t1)

        # =========================================================
        # STAGE K: fc1-T2 (cols 512:790, incl. fixed-up temporal) + fc2-T2
        # =========================================================
        fc1_tile(512, T)
        fc2_tile(512, T)

    nc.compile()
    return nc


# ---------------- host side ----------------
_compiled = {}


def kernel(**inputs):
    x = np.ascontiguousarray(np.asarray(inputs['x'], np.float32))
    qkv_w = np.asarray(inputs['qkv_w'], np.float32)
    proj_w = np.asarray(inputs['proj_w'], np.float32)
    proj_b = np.asarray(inputs['proj_b'], np.float32)
    fc1_w = np.asarray(inputs['fc1_w'], np.float32)
    fc1_b = np.asarray(inputs['fc1_b'], np.float32)
    fc2_w = np.asarray(inputs['fc2_w'], np.float32)
    fc2_b = np.asarray(inputs['fc2_b'], np.float32)
    g = np.asarray(inputs['ln2_g'], np.float32)
    bb_ = np.asarray(inputs['ln2_b'], np.float32)

    import ml_dtypes
    bf16 = ml_dtypes.bfloat16

    # fold LN gamma into the weights, LN beta into effective output biases
    qkv_wg = qkv_w * g[None, :]
    fc1_wg = fc1_w * g[None, :]
    qkv_beta = qkv_wg @ bb_                     # [2304]
    fc1_b_eff = fc1_b + fc1_wg @ bb_            # [3072]

    qkvWt = np.ascontiguousarray(qkv_wg.T).astype(bf16)    # [768, 2304]
    projWt = np.ascontiguousarray(proj_w.T).astype(bf16)   # [768, 768]
    fc1Wt = np.ascontiguousarray(fc1_wg.T).astype(bf16)    # [768, 3072]
    fc2Wt = np.ascontiguousarray(fc2_w.T).astype(bf16)     # [3072, 768]

    # bias blob [128, BB_W] fp32
    bblob = np.zeros((128, BB_W), np.float32)
    bblob[:, BB_QB:BB_QB + 6] = qkv_beta[0:C].reshape(6, 128).T
    bblob[:, BB_KB:BB_KB + 6] = qkv_beta[C:2 * C].reshape(6, 128).T
    bblob[:, BB_VB:BB_VB + 6] = qkv_beta[2 * C:3 * C].reshape(6, 128).T
    pb_eff = proj_b + proj_w @ qkv_beta[2 * C:3 * C]   # fold beta_v through proj
    bblob[:, BB_PB:BB_PB + 6] = pb_eff.reshape(6, 128).T
    bblob[:, BB_F1B:BB_F1B + 24] = fc1_b_eff.reshape(24, 128).T
    bblob[:, BB_F2B:BB_F2B + 6] = fc2_b.reshape(6, 128).T
    bblob[:, BB_EPS] = EPS
    bblob[13:22, BB_S2B] = -1e4

    # consts blob [128, CB_W] bf16 (per-core: mask row differs by parity)
    def make_cblob(even):
        cbm = np.zeros((128, CB_W), np.float32)
        cbm[:, CB_ONES] = 1.0
        for hh in range(H):
            cbm[hh, CB_HSEL + hh * 64:CB_HSEL + (hh + 1) * 64] = 1.0
        for hh in range(H):
            cbm[hh * F:(hh + 1) * F, CB_BD9 + hh] = 1.0
            cbm[hh, CB_BD9T + hh * F:CB_BD9T + (hh + 1) * F] = 1.0
        if even:
            for hh in range(H):
                cbm[0, CB_MASK + hh * F] = 1.0
        cbm[0, CB_ONESROW:CB_ONESROW + 128] = 1.0
        for j in range(F):
            cbm[j, CB_ID9 + j] = 1.0
        for hh in range(H):
            cbm[hh * F:(hh + 1) * F,
                CB_CMASK + hh * 64:CB_CMASK + (hh + 1) * 64] = 1.0
            for j in range(F):
                cbm[hh * F + j, CB_SELT + j] = 1.0
        return cbm.astype(bf16)

    cblob_even = make_cblob(True)
    cblob_odd = make_cblob(False)
    ident = np.zeros((16, 16), np.float32)
    np.fill_diagonal(ident, 1.0)

    in_maps = []
    for core in range(8):
        b, half = core // 2, core % 2
        sp = x[b, F + half * SPH: F + (half + 1) * SPH]      # [780, C]
        tmp = x[b, 0:F]                                       # [9, C]
        pad = np.zeros((1, C), np.float32)
        xT = np.ascontiguousarray(
            np.concatenate([sp, tmp, pad], 0).T).astype(bf16)  # [C, 790]
        in_maps.append(dict(
            xT=xT, qkvWt=qkvWt, projWt=projWt, fc1Wt=fc1Wt, fc2Wt=fc2Wt,
            bblob=bblob, cblob=(cblob_even if half == 0 else cblob_odd),
            ident=ident))

    if 'nc' not in _compiled:
        _compiled['nc'] = build_kernel()
    nc = _compiled['nc']
    res = run_bass_kernel_spmd(nc, in_maps, list(range(8)))
    _compiled['last_result'] = res

    out = np.zeros((B, N, C), np.float32)
    for core in range(8):
        b, half = core // 2, core % 2
        oT = np.asarray(res.results[core]['outT']).astype(np.float32)  # [C, 790]
        if half == 0:
            out[b, 0:F] = oT[:, SPH:SPH + F].T
            out[b, F:F + SPH] = oT[:, 0:SPH].T
        else:
            out[b, F + SPH:N] = oT[:, 0:SPH].T
    return out


if __name__ == '__main__':
    from reference import setup_inputs, reference
    inputs = {k: np.asarray(v) for k, v in setup_inputs().items()}
    out = kernel(**inputs)
    print("kernel ran, out shape", out.shape)
